# revision 1
# baseline (speedup 1.0000x reference)
"""Trainium2 Bass kernel for BoundaryFocalLoss.

Full-input contract: kernel(**inputs) takes the complete arrays
(inputs [128,200000] f32, targets [128,200000] i32, mask [128,200000] f32)
and returns the scalar loss, distributing work over 8 NeuronCores by
sharding the T dimension (each core: all 128 batch rows x 25000 columns,
targets carry a 4/3-column halo for the 7-wide boundary window).

Math (equivalent to the reference):
    E   = exp(x)
    SP  = ln(1+E)             # softplus(x) = relu(x) + ln(1+exp(-|x|))
    Rx  = exp(-SP)            # = 1 - sigmoid(x)
    bce = SP - x*s,  s = 0.025+0.95t   (t' = s sent from host, bf16)
    pt  = exp(-bce)
    ada = 1 - |Rx-0.5|        # = 1 - |sigmoid(x)-0.5|
    F   = (0.75-0.5t)*ada*(1+4*dilate7(trans))*(1-pt)^2*bce
    loss = sum(F*mask)/sum(mask)

Both the ada=(1-za) factor AND the boundary weight W=1+4*d3 are expanded
by linearity into a four-accumulator PE split (PE has ample slack),
removing their multiply chains from DVE:
  sum F = sum ao*bce + 4*sum ao*(d3*bce) - sum (ao*za)*bce - 4*sum (ao*za)*(d3*bce)
with ao = aw*omp2; the final scalar is
  (diag(A1)-diag(A3)) + 4*(diag(A2)-diag(A4)).

Engine assignment (per-tile, N=2500, 3-stage software-pipelined loop so
the in-order engine queues never stall on same-tile cross-engine deps):
    ACT (5): E, SP, Rx, pt, omp2=(1-pt)^2      -- one act-table set
    DVE (9 TT@2x + 4 TS@4x): xq, bce, TR, d1, d2, d3, m1=aw*W,
          lhs=m1*omp2, rhs2=za*bce; aw, W, z=Rx-0.5, za=|z| (bit-and)
    PE:  two PSUM accumulators, diagonals extracted once at the end
Pool/GPSIMD is deliberately unused: its elementwise TensorScalar ops
measure ~13us per [128,2500] tile on HW (~25x the cost-model estimate),
and TensorTensor/abs_max are rejected by the CoreV3 ISA checker anyway.
Measured on HW (per-iteration, device-side repeat loop): ~133us vs the
324.7us baseline.

Inputs are cast host-side to bf16 (x as-is, targets to s=0.025+0.95t),
halving HBM traffic and removing all on-device staging copies.
"""

import numpy as np
from contextlib import ExitStack

P = 128          # partitions == batch rows
N_CORES = 8
HALO_L, HALO_R = 4, 3
HALO = HALO_L + HALO_R


def _build_program_v4(T_shard, N, with_mask, CH=125, repeat=1, use_pool=False, za_act=False, z_act=False):
    """Build + compile the single-core Bass program (SPMD across cores).

    repeat>1 wraps the whole tile loop in a device-side For_i so the body
    executes `repeat` times per launch — used only for wall-clock timing.
    """
    from contextlib import nullcontext
    import concourse.bacc as bacc
    import concourse.tile as tile
    import concourse.mybir as mybir

    dt = mybir.dt
    Alu = mybir.AluOpType
    Act = mybir.ActivationFunctionType

    NT = T_shard // N
    assert NT * N == T_shard
    assert N % CH == 0 and N % 2 == 0
    # smaller edge tiles shorten pipeline fill/drain latency (sizes must
    # stay even for the uint32-pair bitcast and divisible by CH)
    if NT >= 4 and N == 2500 and CH == 125:
        ramp = [500, 750, 1250]         # sums to one full tile
        sizes = ramp + [N] * (NT - 2) + ramp[::-1]
        assert sum(sizes) == T_shard
    elif NT >= 4 and N % (2 * CH) == 0 and (N // 2) % 2 == 0:
        h = N // 2
        sizes = [h, h] + [N] * (NT - 2) + [h, h]
    else:
        sizes = [N] * NT
    offs = [sum(sizes[:k]) for k in range(len(sizes))]

    # The stock act-table-load pass assigns each activation the FIRST
    # act_info set containing its function, which thrashes ~2.7us table
    # reloads. All functions used here (Exp/Ln/Square) co-reside in
    # natural_log_exp_and_others, so strip them from every other set's
    # advertised contents; the pass then lands everything on that one set
    # and a single load suffices.
    import concourse.hw_specs as hw_specs
    import bass_rust as _bass_rust

    _ONE_SET = "natural_log_exp_and_others"
    _USED = {
        mybir.ActivationFunctionType.Exp,
        mybir.ActivationFunctionType.Ln,
        mybir.ActivationFunctionType.Square,
        mybir.ActivationFunctionType.Copy,
        mybir.ActivationFunctionType.Identity,
    }

    class _OneActSetBacc(bacc.Bacc):
        def insert_act_table_loads(self):
            has_activation = any(
                isinstance(i, mybir.InstActivation)
                for b in self.main_func.blocks
                for i in b.instructions
            )
            if not has_activation:
                return
            tables = [
                (name, (funcs if name == _ONE_SET else funcs - _USED))
                for name, funcs in hw_specs.get_activation_tables(self.m.arch).items()
            ]
            _bass_rust.insert_act_table_loads(self, tables)

    nc = _OneActSetBacc("TRN2", target_bir_lowering=False, debug=False)

    if za_act:
        # Abs activation bias must be a const AP; only {0,1} pre-registered.
        _cb = nc.alloc_sbuf_tensor("const-f32-neghalf", [128, 1], dt.float32)
        nc.gpsimd.memset(_cb.ap(), -0.5)
        nc.const_aps.aps[(dt.float32, -0.5)] = _cb.ap()
        nc.all_engine_barrier()

    x_d = nc.dram_tensor("x", [P, T_shard], dt.bfloat16, kind="ExternalInput").ap()
    t_d = nc.dram_tensor("t", [P, T_shard + HALO], dt.bfloat16, kind="ExternalInput").ap()
    eye_d = nc.dram_tensor("eye", [P, P], dt.float32, kind="ExternalInput").ap()
    if with_mask:
        m_d = nc.dram_tensor("m", [P, T_shard], dt.float32, kind="ExternalInput").ap()
    out_d = nc.dram_tensor("out", [P, 2], dt.float32, kind="ExternalOutput").ap()

    with tile.TileContext(nc) as tc, ExitStack() as ctx:
        io = ctx.enter_context(tc.tile_pool(name="io", bufs=2))
        if with_mask:
            iom = ctx.enter_context(tc.tile_pool(name="iom", bufs=3))
        val = ctx.enter_context(tc.tile_pool(name="val", bufs=2))
        val3 = ctx.enter_context(tc.tile_pool(name="val3", bufs=3))
        singles = ctx.enter_context(tc.tile_pool(name="singles", bufs=1))
        psum = ctx.enter_context(tc.tile_pool(name="psum", bufs=1, space="PSUM"))

        eye_sb = singles.tile([P, P], dt.float32)
        nc.sync.dma_start(eye_sb[:], eye_d[:])
        out_sb = singles.tile([P, 2], dt.float32)
        nc.vector.memset(out_sb[:], 0.0)
        if with_mask:
            ms = singles.tile([P, 16], dt.float32)
        acc1 = psum.tile([P, CH], dt.float32)
        acc2 = psum.tile([P, CH], dt.float32)
        acc3 = psum.tile([P, CH], dt.float32)
        acc4 = psum.tile([P, CH], dt.float32)

        last = len(sizes) - 1
        n_tiles = len(sizes)
        rep_cm = tc.For_i(0, repeat, 1) if repeat > 1 else nullcontext()
        with rep_cm:
          # 3-stage software pipeline: every engine's per-iteration ops have
          # dependencies that are >= 1 tile old, so the in-order engine
          # queues never stall on same-tile cross-engine chains.
          #   A(i):   loads; E,SP,Rx [ACT]; xq,TR,d1,d2,d3,bce [DVE]; aw,z [Pool]
          #   B(i-1): pt,omp2 [ACT]; za [DVE]; W [Pool]
          #   C(i-2): m1,lhs,rhs2 [DVE]; 2x matmuls [PE]
          st = {}
          for k in range(n_tiles + 2):
            if k >= 2:  # ---- stage C for tile k-2 --------------------
                l = k - 2
                Nc = sizes[l]
                s = st[l]
                ao = val.tile([P, Nc], dt.bfloat16, tag="xq")
                nc.vector.tensor_tensor(ao[:], s["aw"][:], s["omp2"][:], Alu.mult)
                lhs2 = val.tile([P, Nc], dt.bfloat16, tag="d1")
                nc.vector.tensor_tensor(lhs2[:], ao[:], s["za"][:], Alu.mult)
                rhs1 = s["bce"]
                if with_mask:
                    bm = val.tile([P, Nc], dt.bfloat16, tag="bm")
                    nc.vector.tensor_tensor(bm[:], s["bce"][:], s["m"][:], Alu.mult)
                    rhs1 = bm
                    nc.vector.tensor_reduce(
                        ms[:, l:l + 1], s["m"][:], axis=mybir.AxisListType.X,
                        op=Alu.add)
                d3b = val.tile([P, Nc], dt.bfloat16, tag="d2")
                nc.vector.tensor_tensor(d3b[:], s["d3"][:], rhs1[:], Alu.mult)
                n_chunks = Nc // CH
                for c in range(n_chunks):
                    s0 = c * CH
                    stt = (l == 0 and c == 0)
                    spp = (l == last and c == n_chunks - 1)
                    nc.tensor.matmul(acc1[0:CH, 0:CH], ao[:, s0:s0 + CH],
                                     rhs1[:, s0:s0 + CH], start=stt, stop=spp)
                    nc.tensor.matmul(acc2[0:CH, 0:CH], ao[:, s0:s0 + CH],
                                     d3b[:, s0:s0 + CH], start=stt, stop=spp)
                    nc.tensor.matmul(acc3[0:CH, 0:CH], lhs2[:, s0:s0 + CH],
                                     rhs1[:, s0:s0 + CH], start=stt, stop=spp)
                    nc.tensor.matmul(acc4[0:CH, 0:CH], lhs2[:, s0:s0 + CH],
                                     d3b[:, s0:s0 + CH], start=stt, stop=spp)
                del st[l]

            if k < n_tiles:  # ---- stage A for tile k -----------------
                i, c0, Nc = k, offs[k], sizes[k]
                s = st.setdefault(i, {})
                x_t = io.tile([P, Nc], dt.bfloat16, tag="x")
                nc.sync.dma_start(x_t[:], x_d[:, c0:c0 + Nc])
                t_t = io.tile([P, Nc + HALO], dt.bfloat16, tag="t")
                nc.sync.dma_start(t_t[:], t_d[:, c0:c0 + Nc + HALO])
                if with_mask:
                    m_t = iom.tile([P, Nc], dt.float32, tag="m")
                    nc.sync.dma_start(m_t[:], m_d[:, c0:c0 + Nc])
                    s["m"] = m_t
                t_c = t_t[:, HALO_L:HALO_L + Nc]

                # Pool: aw first (ready at iteration start)
                aw = val3.tile([P, Nc], dt.bfloat16, tag="aw")
                (nc.gpsimd if use_pool else nc.vector).tensor_scalar(
                    aw[:], t_c, -0.5263157894736842, 0.7631578947368421,
                    Alu.mult, Alu.add)
                s["aw"] = aw

                E = val.tile([P, Nc], dt.bfloat16, tag="E")
                nc.scalar.activation(E[:], x_t[:], Act.Exp)
                SP = val.tile([P, Nc], dt.bfloat16, tag="SP")
                nc.scalar.activation(SP[:], E[:], Act.Ln, bias=1.0)
                Rx = val.tile([P, Nc], dt.bfloat16, tag="Rx")
                nc.scalar.activation(Rx[:], SP[:], Act.Exp, scale=-1.0)

                if za_act:
                    za = val3.tile([P, Nc], dt.bfloat16, tag="za")
                    nc.scalar.activation(za[:], Rx[:], Act.Abs, bias=-0.5)
                    s["za"] = za
                elif z_act:
                    z = val.tile([P, Nc], dt.bfloat16, tag="z")
                    nc.scalar.activation(z[:], Rx[:], Act.Copy, bias=-0.5)
                    s["z"] = z
                else:
                    z = val.tile([P, Nc], dt.bfloat16, tag="z")
                    (nc.gpsimd if use_pool else nc.vector).tensor_scalar(
                        z[:], Rx[:], -0.5, 0.0, Alu.add, Alu.add)
                    s["z"] = z

                xq = val.tile([P, Nc], dt.bfloat16, tag="xq")
                nc.vector.tensor_tensor(xq[:], x_t[:], t_c, Alu.mult)
                TR = val.tile([P, Nc + 6], dt.bfloat16, tag="TR")
                nc.vector.tensor_tensor(
                    TR[:], t_t[:, 1:Nc + 7], t_t[:, 0:Nc + 6], Alu.not_equal)
                d1 = val.tile([P, Nc + 5], dt.bfloat16, tag="d1")
                nc.vector.tensor_tensor(
                    d1[:], TR[:, 0:Nc + 5], TR[:, 1:Nc + 6], Alu.max)
                d2 = val.tile([P, Nc + 3], dt.bfloat16, tag="d2")
                nc.vector.tensor_tensor(
                    d2[:], d1[:, 0:Nc + 3], d1[:, 2:Nc + 5], Alu.max)
                d3 = val3.tile([P, Nc], dt.bfloat16, tag="d3")
                nc.vector.tensor_tensor(
                    d3[:], d2[:, 0:Nc], d2[:, 3:Nc + 3], Alu.max)
                s["d3"] = d3
                bce = val3.tile([P, Nc], dt.bfloat16, tag="bce")
                nc.vector.tensor_tensor(bce[:], SP[:], xq[:], Alu.subtract)
                s["bce"] = bce

            if 1 <= k <= n_tiles:  # ---- stage B for tile k-1 ---------
                j = k - 1
                Nc = sizes[j]
                s = st[j]
                pt = val.tile([P, Nc], dt.bfloat16, tag="E")
                nc.scalar.activation(pt[:], s["bce"][:], Act.Exp, scale=-1.0)
                omp2 = val.tile([P, Nc], dt.bfloat16, tag="omp2")
                nc.scalar.activation(omp2[:], pt[:], Act.Square, bias=1.0, scale=-1.0)
                s["omp2"] = omp2
                if not za_act:
                    za = val.tile([P, Nc], dt.bfloat16, tag="TR")
                    nc.vector.tensor_scalar(
                        za[:].bitcast(dt.uint32), s["z"][:].bitcast(dt.uint32),
                        0x7FFF7FFF, None, Alu.bitwise_and)
                    s["za"] = za

        # ---- tail: diag(acc1) - diag(acc2) holds per-column sums ------
        a1sb = singles.tile([P, CH], dt.float32)
        nc.vector.tensor_copy(a1sb[0:CH, :], acc1[0:CH, 0:CH])
        s13 = singles.tile([P, CH], dt.float32)
        nc.vector.tensor_tensor(
            s13[0:CH, :], a1sb[0:CH, :], acc3[0:CH, 0:CH], Alu.subtract)
        a2sb = singles.tile([P, CH], dt.float32)
        nc.vector.tensor_copy(a2sb[0:CH, :], acc2[0:CH, 0:CH])
        s24 = singles.tile([P, CH], dt.float32)
        nc.vector.tensor_tensor(
            s24[0:CH, :], a2sb[0:CH, :], acc4[0:CH, 0:CH], Alu.subtract)
        accsb = singles.tile([P, CH], dt.float32)
        nc.vector.scalar_tensor_tensor(
            accsb[0:CH, :], s24[0:CH, :], 4.0, s13[0:CH, :], Alu.mult, Alu.add)
        diag = singles.tile([P, CH], dt.float32)
        nc.vector.tensor_tensor(
            diag[0:CH, :], accsb[0:CH, :], eye_sb[0:CH, 0:CH], Alu.mult)
        nc.vector.tensor_reduce(
            out_sb[0:CH, 0:1], diag[0:CH, :], axis=mybir.AxisListType.X, op=Alu.add)
        if with_mask:
            nc.vector.tensor_reduce(
                out_sb[:, 1:2], ms[:], axis=mybir.AxisListType.X, op=Alu.add)
        nc.sync.dma_start(out_d[:], out_sb[:])

    nc.compile()
    return nc


_PROGRAM_CACHE = {}


def _get_program(T_shard, N, with_mask):
    key = (T_shard, N, with_mask)
    if key not in _PROGRAM_CACHE:
        _PROGRAM_CACHE[key] = _build_program_v4(T_shard, N, with_mask)
    return _PROGRAM_CACHE[key]


def _host_inputs(inputs, targets):
    import ml_dtypes
    bf16 = ml_dtypes.bfloat16
    x = np.asarray(inputs, dtype=np.float32).astype(bf16)
    tf = np.asarray(targets, dtype=np.float32)
    t = (0.025 + 0.95 * tf).astype(bf16)
    return x, t


def kernel(inputs, targets, mask):
    from concourse.bass_utils import run_bass_kernel_spmd

    x, t = _host_inputs(inputs, targets)
    m = np.ascontiguousarray(np.asarray(mask, dtype=np.float32))
    Bq, T = x.shape
    assert Bq == P and T % N_CORES == 0
    T_shard = T // N_CORES
    N = 2500
    ones_mask = bool(m.min() == 1.0 and m.max() == 1.0)

    nc = _get_program(T_shard, N, with_mask=not ones_mask)

    t_pad = np.pad(t, ((0, 0), (HALO_L, HALO_R)), mode="edge")
    eye = np.eye(P, dtype=np.float32)
    in_maps = []
    for c in range(N_CORES):
        lo = c * T_shard
        im = {
            "x": np.ascontiguousarray(x[:, lo:lo + T_shard]),
            "t": np.ascontiguousarray(t_pad[:, lo:lo + T_shard + HALO]),
            "eye": eye,
        }
        if not ones_mask:
            im["m"] = np.ascontiguousarray(m[:, lo:lo + T_shard])
        in_maps.append(im)

    res = run_bass_kernel_spmd(nc, in_maps, core_ids=list(range(N_CORES)))
    outs = [r["out"] for r in res.results]

    loss = float(sum(o[:, 0].astype(np.float64).sum() for o in outs))
    if ones_mask:
        msum = float(Bq) * float(T)
    else:
        msum = float(sum(o[:, 1].astype(np.float64).sum() for o in outs))
    if msum <= 0.0:
        return np.float32(0.0)
    return np.float32(loss / msum)

